# revision 1
# baseline (speedup 1.0000x reference)
"""Trainium2 Bass kernel for nn_CSAModule_47768626266174.

Mathematical structure of the reference:

    S    = softmax(attn, axis=-1)                # [C, T, T]
    out  = base + sigma * einsum('bft,ct->bcf', inputs, S.mean(axis=-1))
    base = inputs.mean(-1)[:, None, :]

``S.mean(axis=-1)`` averages over the *same* axis the softmax normalizes,
so it is exactly 1/T for every (c, t) — independent of the attention
contents, the conv weights, and the labels.  Hence

    out[b, c, f] = (1 + sigma) * mean_t inputs[b, f, t]

for every class c.  This identity holds for all finite inputs (softmax is
shift-normalized, rows sum to 1), so the kernel only needs to read
``inputs`` once, reduce over T, scale by (1 + sigma), and broadcast over
the class dim.  That is the true memory roofline of this module.

Sharding: data-parallel over batch B — each of the 8 cores reduces its
8-item chunk; no collectives.  Output chunks are concatenated on host.

Raw Bass (not Tile): this container's walrus build encodes at most ONE
semaphore wait per instruction, which rejects Tile's kernel-tail drain.
Standalone wait_ge instructions carry one condition each; anything
needing several predecessors gets several wait_ge's in front.

Per-core dataflow.  The critical path is the serialized DMA data stream
(~2.1 MB at ~360 GB/s); every other stage is pipelined per batch item
behind it, so the post-stream tail is just the last item's short chain:

  sync  : per-item input DMAs (per-DMA semaphores — dynamic HW queues
          complete out of order; the last item is loaded in two halves so
          the tail reduce is half-size), one store of y as [C, B, F]
  gpsimd: sigma DMA on SWDGE (keeps the HWDGE path free for x),
          ident_s = diag((1+sigma)/T) in one affine_select — all early
  vector: s1 = (1+sigma)/T, T-reduces (594 ns/item < 728 ns arrival),
          the last item's second-half reduce and PSUM->SBUF copy
  scalar: the last item's FIRST-half reduce (activation w/ accum_out,
          slotted between copies) so DVE reaches the critical final
          half-reduce with no backlog; per finished item, the PSUM ->
          SBUF copy of its y rows into yt [C, B*F] (compute engines can
          only address partition bases 0/32/64/96, so items advance
          along the free axis)
  tensor: K=1 matmul broadcasting s1 across partitions into psc; then per
          item b a small matmul into its own PSUM bank:
          pt_b = (sums[:, b] bcast over C).T @ ident_s  ([C, F] = y rows)
          (the tail item as two PSUM-accumulating half matmuls)
"""

from contextlib import ExitStack

import numpy as np

B, F, T, C = 64, 128, 512, 10
N_CORES = 8
BPC = B // N_CORES  # batch items per core

_NC_CACHE = None


def _build_bass():
    """Build the per-core Bass module (SPMD: same program on all cores)."""
    global _NC_CACHE
    if _NC_CACHE is not None:
        return _NC_CACHE

    import concourse.bass as bass
    import concourse.mybir as mybir

    fp32 = mybir.dt.float32
    # Bass.__init__ unconditionally memsets four const-AP tiles on the
    # Pool engine BEFORE the program start barrier; Pool is the last
    # engine to reach that barrier, so they delay every engine's release
    # by ~0.3 us.  None of them have readers in this kernel (walrus flags
    # them as dead), so skip their emission.  memset is re-bound into
    # BassEitherVectorEngine at class-definition time — patch there.
    _orig_memset = bass.BassEitherVectorEngine.memset

    def _memset_skip_dead_consts(self, ap, constant):
        tensor = getattr(ap, "tensor", None)
        if tensor is not None and getattr(tensor, "name", "").startswith(
            "const-"
        ):
            return None
        return _orig_memset(self, ap, constant)

    # The start barrier emitted at the end of Bass.__init__ only orders
    # those const-tile memsets against the program body; with the memsets
    # gone, every cross-engine dependency in this kernel is already
    # semaphore-guarded (CoreSim's race detector verifies), so skip it
    # too — it costs ~0.5 us before the first DMA can issue.
    _orig_barrier = bass.Bass.all_engine_barrier

    def _skip_barrier(self, *, sem_only: bool = False):
        return None

    bass.BassEitherVectorEngine.memset = _memset_skip_dead_consts
    bass.Bass.all_engine_barrier = _skip_barrier
    try:
        nc = bass.Bass()
    finally:
        bass.BassEitherVectorEngine.memset = _orig_memset
        bass.Bass.all_engine_barrier = _orig_barrier

    x = nc.dram_tensor("x", [BPC, F, T], fp32, kind="ExternalInput")
    sig = nc.dram_tensor("sig", [1, 1], fp32, kind="ExternalInput")
    y = nc.dram_tensor("y", [BPC, C, F], fp32, kind="ExternalOutput")

    with ExitStack() as ctx:
        e = ctx.enter_context
        xt = e(nc.sbuf_tensor("xt", [128, BPC * T], fp32))
        # SPLIT items are loaded/reduced in two halves so the tail reduce
        # is half-size and starts earlier; the PE recombines each pair via
        # PSUM accumulation.  Only the last item: each dma_start costs
        # ~650 ns of issue time vs 364 ns of data time for a half chunk,
        # so more splits stall the stream on descriptor generation.
        SPLIT = [BPC - 1]
        # A few spare columns for split items' partial sums.
        sums = e(nc.sbuf_tensor("sums", [128, BPC + 4], fp32))
        ident_s = e(nc.sbuf_tensor("ident_s", [128, 128], fp32))
        sg = e(nc.sbuf_tensor("sg", [1, 1], fp32))
        s1 = e(nc.sbuf_tensor("s1", [1, 1], fp32))
        ones_row = e(nc.sbuf_tensor("ones_row", [1, 128], fp32))
        scale_col = e(nc.sbuf_tensor("scale_col", [128, 1], fp32))
        # [C partitions, BPC*F free]: per-item copies land at free-dim
        # offsets (compute engines may only start at partition 0/32/64/96).
        yt = e(nc.sbuf_tensor("yt", [C, BPC * F], fp32))
        # psc is allocated and immediately freed: its bank is reused by
        # pts[0].  Safe because the first per-item matmul waits for the
        # scale_col copy, after which psc is dead.
        psc_cm = nc.psum_tensor("psc", [128, 1], fp32)
        psc = psc_cm.__enter__()
        psc_cm.__exit__(None, None, None)
        # One PSUM bank per item: matmul outputs must start at partition
        # 0/32/64, and bank separation means the PE write of item b+1
        # never touches the bank ACT is reading for item b.
        pts = [e(nc.psum_tensor(f"pt{b}", [C, 128], fp32)) for b in range(BPC)]

        # Load plan: (item, sums-column, t-range, semaphore, reduce
        # engine).  One semaphore per DMA: dynamic HW queues complete out
        # of order.  The split item's FIRST half reduces on ACT, so DVE
        # reaches the critical final half-reduce with no backlog.  Two
        # chunks measured best: each extra chunk adds a full matmul
        # (~213 ns) to the serial PE tail but saves only ~146 ns of
        # reduce time.
        H = T // 2
        TAIL_CHUNKS = [(0, H, "act"), (H, T, "dve")]
        loads = []
        extra_col = BPC
        for b in range(BPC):
            if b in SPLIT:
                for i, (t0, t1, eng) in enumerate(TAIL_CHUNKS):
                    col = b if i == 0 else extra_col
                    if i > 0:
                        extra_col += 1
                    loads.append(
                        (b, col, t0, t1, e(nc.semaphore(f"xld{b}_{i}")), eng)
                    )
            else:
                loads.append((b, b, 0, T, e(nc.semaphore(f"xld{b}")), "dve"))

        dump = e(nc.sbuf_tensor("dump", [128, H], fp32))

        sig_sem = e(nc.semaphore("sig_sem"))
        store_sem = e(nc.semaphore("store_sem"))
        dve_sem = e(nc.semaphore("dve_sem"))
        act_sem = e(nc.semaphore("act_sem"))
        act_red_sem = e(nc.semaphore("act_red_sem"))
        pe_sem = e(nc.semaphore("pe_sem"))
        pool_sem = e(nc.semaphore("pool_sem"))

        block = e(nc.Block())

        # Precomputed milestones (block bodies trace immediately, so no
        # cross-block mutable state).
        # dve_sem: 1 ones_row, 2 s1, 3 scale_col, then one per DVE reduce
        # in load order, then +1 for the last item's PSUM->SBUF copy.
        dve_red_ms = {}
        act_red_ms = {}
        dve_n = 3
        act_red_n = 0
        for b, col, t0, t1, sem, eng in loads:
            if eng == "dve":
                dve_n += 1
                dve_red_ms[(b, col)] = dve_n
            else:
                act_red_n += 1
                act_red_ms[(b, col)] = act_red_n
        DVE_COPY_MS = dve_n + 1
        # pe_sem: 1 psc, then one per matmul in item order (split item's
        # halves are consecutive accumulating matmuls).
        mm_plan = []  # (item, col, start, stop, wait_engine, wait_value)
        pe_n = 1
        mm_ms = {}
        for b in range(BPC):
            cols = [(col, eng) for (bb, col, t0, t1, s, eng) in loads if bb == b]
            for i, (col, eng) in enumerate(cols):
                wait = (
                    ("dve", dve_red_ms[(b, col)])
                    if eng == "dve"
                    else ("act", act_red_ms[(b, col)])
                )
                pe_n += 1
                mm_plan.append(
                    (b, col, i == 0, i == len(cols) - 1, wait[0], wait[1])
                )
            mm_ms[b] = pe_n

        @block.sync
        def _(sync):
            for b, col, t0, t1, sem, eng in loads:
                sync.dma_start(
                    xt[:, b * T + t0 : b * T + t1], x[b, :, t0:t1]
                ).then_inc(sem, 16)
            sync.wait_ge(act_sem, BPC - 1)  # yt columns 0..BPC-2 copied
            sync.wait_ge(dve_sem, DVE_COPY_MS)  # last yt column (DVE)
            sync.dma_start(
                y[:, :, :].rearrange("b c f -> c b f"),
                yt[:, :].rearrange("c (b f) -> c b f", f=F),
            ).then_inc(store_sem, 16)
            sync.wait_ge(store_sem, 16)

        @block.gpsimd
        def _(gpsimd):
            # SWDGE load of sigma — the HWDGE descriptor path stays free
            # for the x stream.
            gpsimd.dma_start(sg[:, :], sig[:, :]).then_inc(sig_sem, 16)
            # ident_s = diag((1+sigma)/T) in a single op: select between a
            # step-0 broadcast of scale_col and 0.0.
            gpsimd.wait_ge(dve_sem, 3)  # scale_col ready
            gpsimd.affine_select(
                out=ident_s[:, :],
                in_=scale_col[:, :].broadcast_to((128, 128)),
                compare_op=mybir.AluOpType.is_equal,
                fill=0.0,
                base=0,
                pattern=[[-1, 128]],
                channel_multiplier=1,
            ).then_inc(pool_sem, 1)  # p1

        @block.vector
        def _(vector):
            vector.memset(ones_row[:, :], 1.0).then_inc(dve_sem, 1)
            vector.wait_ge(sig_sem, 16)
            # s1 = sigma/T + 1/T = (1+sigma)/T
            vector.tensor_scalar(
                out=s1[:, :],
                in0=sg[:, :],
                scalar1=1.0 / T,
                scalar2=1.0 / T,
                op0=mybir.AluOpType.mult,
                op1=mybir.AluOpType.add,
            ).then_inc(dve_sem, 1)
            vector.wait_ge(pe_sem, 1)  # psc ready
            vector.tensor_copy(scale_col[:, :], psc[:, :]).then_inc(dve_sem, 1)
            for b, col, t0, t1, sem, eng in loads:
                if eng != "dve":
                    continue
                vector.wait_ge(sem, 16)
                vector.reduce_sum(
                    out=sums[:, col : col + 1],
                    in_=xt[:, b * T + t0 : b * T + t1],
                    axis=mybir.AxisListType.X,
                ).then_inc(dve_sem, 1)
            # Last item's PSUM -> SBUF copy on DVE (free after its final
            # reduce, and its copy is faster than ACT's).
            vector.wait_ge(pe_sem, mm_ms[BPC - 1])
            vector.tensor_copy(
                yt[:, (BPC - 1) * F : BPC * F], pts[BPC - 1][:, :]
            ).then_inc(dve_sem, 1)

        @block.tensor
        def _(tensor):
            tensor.wait_ge(dve_sem, 2)  # ones_row + s1
            # psc[p, 0] = (1+sigma)/T on every partition (K=1 matmul).
            tensor.matmul(
                psc[:, :], ones_row[:, :], s1[:, :], start=True, stop=True
            ).then_inc(pe_sem, 1)
            tensor.wait_ge(pool_sem, 1)  # ident_s ready
            # Per-item matmuls, issued as each (partial) reduce lands:
            # pt_b[c, f] = sums[f, b] * (1+sigma)/T.  lhsT is the item's
            # sums column broadcast over classes via one step-0 free dim;
            # the sigma scale rides the diagonal matrix; split items
            # accumulate their halves in PSUM.
            for b, col, is_start, is_stop, weng, wval in mm_plan:
                if weng == "dve":
                    tensor.wait_ge(dve_sem, wval)
                else:
                    tensor.wait_ge(act_red_sem, wval)
                tensor.matmul(
                    pts[b][:, :],
                    sums[:, col : col + 1].broadcast_to((128, C)),
                    ident_s[:, :],
                    start=is_start,
                    stop=is_stop,
                ).then_inc(pe_sem, 1)

        @block.scalar
        def _(scalar):
            # Per-item PSUM -> SBUF copies on the otherwise idle ACT
            # engine (the last item's copy runs on DVE instead), with the
            # split item's first-half reduce slotted in between: it must
            # come after enough copies that they are not delayed, but
            # before ACT goes idle waiting on late matmuls.
            act_loads = [
                ld for ld in loads if ld[5] == "act"
            ]
            for b in range(BPC - 1):
                if b == BPC - 3:
                    for bb, col, t0, t1, sem, eng in act_loads:
                        scalar.wait_ge(sem, 16)
                        scalar.activation(
                            out=dump[:, :],
                            in_=xt[:, bb * T + t0 : bb * T + t1],
                            func=mybir.ActivationFunctionType.Copy,
                            accum_out=sums[:, col : col + 1],
                        ).then_inc(act_red_sem, 1)
                scalar.wait_ge(pe_sem, mm_ms[b])
                scalar.activation(
                    out=yt[:, b * F : (b + 1) * F],
                    in_=pts[b][:, :],
                    func=mybir.ActivationFunctionType.Copy,
                ).then_inc(act_sem, 1)

    _NC_CACHE = nc
    return nc


def run_spmd(inputs_arr: np.ndarray, sigma_arr: np.ndarray, trace: bool = False):
    """Shard over batch, run on 8 cores, gather. Returns (out, results_obj)."""
    from concourse import bass_utils

    nc = _build_bass()

    x_full = np.ascontiguousarray(np.asarray(inputs_arr, dtype=np.float32))
    assert x_full.shape == (B, F, T), x_full.shape
    sig = np.asarray(sigma_arr, dtype=np.float32).reshape(1, 1)

    in_maps = [
        {"x": x_full[k * BPC : (k + 1) * BPC], "sig": sig} for k in range(N_CORES)
    ]
    res = bass_utils.run_bass_kernel_spmd(
        nc, in_maps, core_ids=list(range(N_CORES)), trace=trace
    )
    out = np.concatenate([r["y"] for r in res.results], axis=0)
    return out, res


def kernel(**inputs) -> np.ndarray:
    out, _ = run_spmd(inputs["inputs"], inputs["sigma"])
    return out



# revision 33
# speedup vs baseline: 1.0791x; 1.0791x over previous
"""Trainium2 Bass kernel for nn_CSAModule_47768626266174.

Mathematical structure of the reference:

    S    = softmax(attn, axis=-1)                # [C, T, T]
    out  = base + sigma * einsum('bft,ct->bcf', inputs, S.mean(axis=-1))
    base = inputs.mean(-1)[:, None, :]

``S.mean(axis=-1)`` averages over the *same* axis the softmax normalizes,
so it is exactly 1/T for every (c, t) — independent of the attention
contents, the conv weights, and the labels.  Hence

    out[b, c, f] = (1 + sigma) * mean_t inputs[b, f, t]

for every class c, so the kernel only needs to read ``inputs`` once,
reduce over T, scale by (1 + sigma)/T, and broadcast over the class dim.

Sharding: data-parallel over batch B — each of the 8 cores reduces its
8-item chunk; no collectives.  Output chunks are concatenated on host.

Per-core dataflow.  The critical path is the serialized DMA data stream
(~2.1 MB at 360 GB/s), then the tail chain of the LAST-arriving chunk:
dma-sem (900) -> reduce -> matmul -> PSUM->SBUF copy -> store launch
(HWDGE 625 + DGE 650) -> transfer -> dma-sem (900).  Design notes:

  * SP's 5-instruction register preamble is skipped (-250 ns stream
    start; SP only issues DMAs / waits, which never read those regs).
  * 9 load DMAs (HWDGE is 625 ns per DMA; a 10th would stall the
    stream): items 0-6 whole, item 7 in halves so the last chunk's
    reduce is half-length.  Each load has its own semaphore (dynamic
    queues complete out of order).
  * DVE reduces items 0-5 and 7a/7b; item 6 reduces on ACT via
    activation+accumulate, so DVE is FREE exactly when 7a/7b's
    semaphores fire and the last reduces run semaphore-paced.
  * Item 7's sums are written in bf16 and its two matmuls use a bf16
    scaled-identity: 1 PE cycle/row instead of fp32's 4 (53 vs 213 ns).
    ~0.4% rounding on 1/8 of the output, vs a 2e-2 rel-err budget.
  * PE matmul order: items 0-5, then 7a(start), 6, 7b(stop).  mm7a's
    input is ready before mm6's (ACT's accum-read chain is slow) and
    mm7b's reduce lands last, so this keeps PE busy without delaying
    mm7b.  pts[6]/pts[7] are separate PSUM banks so the interleaved
    accumulation groups don't interact.  PE uses STANDALONE wait_ge
    (not waits attached to the matmul): the busy sequencer pins
    pe_busy_start so the PE p-state ramps to full speed (213 ns/mm)
    instead of restarting cold (607 ns/mm) at every matmul.
  * Copies: items 0-5 and 6 on ACT into yt; item 7 on DVE (GPSIMD may
    not touch PSUM; DVE is free after its last reduce).
  * Three HWDGE stores from SP: items 0-3 (after c3), 4-5 (after c5),
    6-7 (after c6+c7, the only store on the critical tail).  SWDGE
    prepare/trigger would launch ~1.2 us faster after the data is
    ready, but this walrus build cannot encode InstTriggerDma
    ("ISA wrong length"), so the HWDGE path it is.
  * The Bass start barrier, its dead const-tile memsets, and the
    Block-exit all-engine barrier are skipped; SP's final waits on the
    three store semaphores keep the program alive until y is in HBM.
"""

from contextlib import ExitStack

import numpy as np

B, F, T, C = 64, 128, 512, 10
N_CORES = 8
BPC = B // N_CORES  # batch items per core
H = T // 2

_NC_CACHE = None


def _build_bass():
    """Build the per-core Bass module (SPMD: same program on all cores)."""
    global _NC_CACHE
    if _NC_CACHE is not None:
        return _NC_CACHE

    import concourse.bass as bass
    import concourse.mybir as mybir

    fp32 = mybir.dt.float32
    bf16 = mybir.dt.bfloat16

    _orig_memset = bass.BassEitherVectorEngine.memset

    def _memset_skip_dead_consts(self, ap, constant):
        tensor = getattr(ap, "tensor", None)
        if tensor is not None and getattr(tensor, "name", "").startswith(
            "const-"
        ):
            return None
        return _orig_memset(self, ap, constant)

    _orig_barrier = bass.Bass.all_engine_barrier

    def _skip_barrier(self, *, sem_only: bool = False):
        return None

    _orig_preamble = bass.BassEngine.preamble

    def _preamble_skip_sp(self):
        if self.engine == mybir.EngineType.SP:
            return None
        return _orig_preamble(self)

    bass.BassEitherVectorEngine.memset = _memset_skip_dead_consts
    bass.Bass.all_engine_barrier = _skip_barrier
    bass.BassEngine.preamble = _preamble_skip_sp
    try:
        nc = bass.Bass()

        x = nc.dram_tensor("x", [BPC, F, T], fp32, kind="ExternalInput")
        sig = nc.dram_tensor("sig", [1, 1], fp32, kind="ExternalInput")
        y = nc.dram_tensor("y", [BPC, C, F], fp32, kind="ExternalOutput")

        with ExitStack() as ctx:
            e = ctx.enter_context
            xt = e(nc.sbuf_tensor("xt", [128, BPC * T], fp32))
            # sums columns: item b -> col b (b=0..6); item 7's halves go
            # to sums16 (bf16) cols 0/1.
            sums = e(nc.sbuf_tensor("sums", [128, 8], fp32))
            sums16 = e(nc.sbuf_tensor("sums16", [128, 2], bf16))
            ident_s = e(nc.sbuf_tensor("ident_s", [128, 128], fp32))
            ident16 = e(nc.sbuf_tensor("ident16", [128, 128], bf16))
            sg = e(nc.sbuf_tensor("sg", [1, 1], fp32))
            s1 = e(nc.sbuf_tensor("s1", [1, 1], fp32))
            ones_row = e(nc.sbuf_tensor("ones_row", [1, 128], fp32))
            scale_col = e(nc.sbuf_tensor("scale_col", [128, 1], fp32))
            yt = e(nc.sbuf_tensor("yt", [C, BPC * F], fp32))
            dump = e(nc.sbuf_tensor("dump", [128, T], fp32))
            # psc is allocated and immediately freed: its bank is reused by
            # pts[0].  Safe because mm0 (the first pts[0] write) waits for
            # ident_s, which waits for the scale_col copy — psc's last read.
            psc_cm = nc.psum_tensor("psc", [128, 1], fp32)
            psc = psc_cm.__enter__()
            psc_cm.__exit__(None, None, None)
            pts = [
                e(nc.psum_tensor(f"pt{b}", [C, 128], fp32))
                for b in range(BPC)
            ]

            # (item, sums-col-key, t0, t1, sem, engine)
            loads = []
            for b in range(6):
                loads.append((b, b, 0, T, e(nc.semaphore(f"x{b}")), "dve"))
            loads.append((6, 6, 0, T, e(nc.semaphore("x6")), "act"))
            loads.append((7, 0, 0, H, e(nc.semaphore("x7a")), "dve16"))
            loads.append((7, 1, H, T, e(nc.semaphore("x7b")), "dve16"))

            sig_sem = e(nc.semaphore("sig_sem"))
            s1_sem = e(nc.semaphore("s1_sem"))
            dve_sem = e(nc.semaphore("dve_sem"))
            pool_sem = e(nc.semaphore("pool_sem"))
            pe_sem = e(nc.semaphore("pe_sem"))
            act_sem = e(nc.semaphore("act_sem"))
            act_red_sem = e(nc.semaphore("act_red_sem"))
            c67_sem = e(nc.semaphore("c67_sem"))
            sa_sem = e(nc.semaphore("sa_sem"))
            sb_sem = e(nc.semaphore("sb_sem"))
            sc_sem = e(nc.semaphore("sc_sem"))

            # dve_sem milestones: 1 = ones_row memset, then one per DVE
            # reduce in program order (s1 rides between r0 and r1 without
            # its own dve_sem inc).
            dve_ms = {}
            n = 1
            for b, col, t0, t1, sem, eng in loads:
                if eng.startswith("dve"):
                    n += 1
                    dve_ms[(b, col)] = n
            # pe_sem: 1 = psc (scale_col broadcast), then the matmuls.
            # Order: items 0-5, 7a(start), 6, 7b(stop).
            mm_order = [(b, b, True, True) for b in range(6)] + [
                (7, 0, True, False),
                (6, 6, True, True),
                (7, 1, False, True),
            ]
            mm_ms = {}
            for i, (b, col, st, sp) in enumerate(mm_order):
                mm_ms[(b, col)] = i + 2
            MM6_DONE = mm_ms[(6, 6)]
            MM7_DONE = mm_ms[(7, 1)]

            # Loads issue in the main body, before the Block's entry
            # branch: SP's first DMA starts at t=0 instead of t=50.
            for b, col, t0, t1, sem, eng in loads:
                nc.sync.dma_start(
                    xt[:, b * T + t0 : b * T + t1], x[b, :, t0:t1]
                ).then_inc(sem, 16)

            block = e(nc.Block())

            @block.sync
            def _(sync):
                # Store A: items 0-3.
                sync.dma_start(
                    y[0:4, :, :].rearrange("b c f -> c b f"),
                    yt[:, 0 : 4 * F].rearrange("c (b f) -> c b f", f=F),
                )._wait_ge(act_sem, 5).then_inc(sa_sem, 16)
                # Store B: items 4-7 — the only store on the critical
                # tail (c4/c5/c6/c7 each bump c67_sem).
                sync.dma_start(
                    y[4:8, :, :].rearrange("b c f -> c b f"),
                    yt[:, 4 * F : 8 * F].rearrange("c (b f) -> c b f", f=F),
                )._wait_ge(c67_sem, 4).then_inc(sb_sem, 16)
                sync.wait_ge(sa_sem, 16)
                sync.wait_ge(sb_sem, 16)

            @block.vector
            def _(vector):
                vector.memset(ones_row[:, :], 1.0).then_inc(dve_sem, 1)
                first = True
                for b, col, t0, t1, sem, eng in loads:
                    if eng == "act":
                        continue
                    if eng == "dve16":
                        with nc.allow_low_precision(
                            "item-7 sums feed a bf16 matmul; ~0.4% rounding"
                        ):
                            red = vector.reduce_sum(
                                out=sums16[:, col : col + 1],
                                in_=xt[:, b * T + t0 : b * T + t1],
                                axis=mybir.AxisListType.X,
                            )
                    else:
                        red = vector.reduce_sum(
                            out=sums[:, col : col + 1],
                            in_=xt[:, b * T + t0 : b * T + t1],
                            axis=mybir.AxisListType.X,
                        )
                    red._wait_ge(sem, 16).then_inc(dve_sem, 1)
                    if first:
                        first = False
                        # s1 = (1+sigma)/T after r0 (sigma's semaphore
                        # fires just after item 1's load).
                        vector.tensor_scalar(
                            out=s1[:, :],
                            in0=sg[:, :],
                            scalar1=1.0 / T,
                            scalar2=1.0 / T,
                            op0=mybir.AluOpType.mult,
                            op1=mybir.AluOpType.add,
                        )._wait_ge(sig_sem, 16).then_inc(s1_sem, 1)
                # Item 7's PSUM -> SBUF copy (GPSIMD may not touch PSUM;
                # DVE is free right after its last reduce).
                vector.tensor_copy(
                    yt[:, 7 * F : 8 * F], pts[7][:, :]
                )._wait_ge(pe_sem, MM7_DONE).then_inc(c67_sem, 1)

            @block.gpsimd
            def _(gpsimd):
                # SWDGE load of sigma — HWDGE path stays free for x.
                gpsimd.dma_start(sg[:, :], sig[:, :]).then_inc(sig_sem, 16)
                # ident_s = diag((1+sigma)/T): select between a broadcast
                # of scale_col and 0.0.
                gpsimd.affine_select(
                    out=ident_s[:, :],
                    in_=scale_col[:, :].broadcast_to((128, 128)),
                    compare_op=mybir.AluOpType.is_equal,
                    fill=0.0,
                    base=0,
                    pattern=[[-1, 128]],
                    channel_multiplier=1,
                )._wait_ge(act_sem, 1).then_inc(pool_sem, 1)
                gpsimd.tensor_copy(ident16[:, :], ident_s[:, :])

            @block.scalar
            def _(scalar):
                # scale_col = (1+sigma)/T on all partitions, via psc.
                scalar.activation(
                    out=scale_col[:, :],
                    in_=psc[:, :],
                    func=mybir.ActivationFunctionType.Copy,
                )._wait_ge(pe_sem, 1).then_inc(act_sem, 1)
                for bidx in range(4):
                    scalar.activation(
                        out=yt[:, bidx * F : (bidx + 1) * F],
                        in_=pts[bidx][:, :],
                        func=mybir.ActivationFunctionType.Copy,
                    )._wait_ge(pe_sem, mm_ms[(bidx, bidx)]).then_inc(act_sem, 1)
                scalar.activation(
                    out=yt[:, 4 * F : 5 * F],
                    in_=pts[4][:, :],
                    func=mybir.ActivationFunctionType.Copy,
                )._wait_ge(pe_sem, mm_ms[(4, 4)]).then_inc(c67_sem, 1)
                # Item 6's reduce: activation+accumulate, slotted before c5
                # (c5's matmul finishes later than item 6's load sem).
                for b, col, t0, t1, sem, eng in loads:
                    if eng != "act":
                        continue
                    scalar.activation(
                        out=dump[:, t0:t1],
                        in_=xt[:, b * T + t0 : b * T + t1],
                        func=mybir.ActivationFunctionType.Copy,
                        accum_out=sums[:, col : col + 1],
                    )._wait_ge(sem, 16).then_inc(act_red_sem, 1)
                scalar.activation(
                    out=yt[:, 5 * F : 6 * F],
                    in_=pts[5][:, :],
                    func=mybir.ActivationFunctionType.Copy,
                )._wait_ge(pe_sem, mm_ms[(5, 5)]).then_inc(c67_sem, 1)
                scalar.activation(
                    out=yt[:, 6 * F : 7 * F],
                    in_=pts[6][:, :],
                    func=mybir.ActivationFunctionType.Copy,
                )._wait_ge(pe_sem, MM6_DONE).then_inc(c67_sem, 1)

            @block.tensor
            def _(tensor):
                # Standalone waits keep PE.SEQ occupied between matmuls,
                # pinning pe_busy_start so the p-state ramps to full speed.
                # psc[p, 0] = (1+sigma)/T on every partition (K=1 matmul).
                tensor.wait_ge(dve_sem, 1)  # ones_row
                tensor.wait_ge(s1_sem, 1)
                tensor.matmul(
                    psc[:, :], ones_row[:, :], s1[:, :], start=True, stop=True
                ).then_inc(pe_sem, 1)
                tensor.wait_ge(pool_sem, 1)  # ident_s ready
                for b, col, is_start, is_stop in mm_order:
                    if (b, col) in dve_ms:
                        tensor.wait_ge(dve_sem, dve_ms[(b, col)])
                    else:
                        tensor.wait_ge(act_red_sem, 1)
                    if b == 7:
                        lhsT = sums16[:, col : col + 1].broadcast_to((128, C))
                        rhs = ident16[:, :]
                    else:
                        lhsT = sums[:, col : col + 1].broadcast_to((128, C))
                        rhs = ident_s[:, :]
                    tensor.matmul(
                        pts[b][:, :],
                        lhsT,
                        rhs,
                        start=is_start,
                        stop=is_stop,
                    ).then_inc(pe_sem, 1)

    finally:
        bass.BassEitherVectorEngine.memset = _orig_memset
        bass.Bass.all_engine_barrier = _orig_barrier
        bass.BassEngine.preamble = _orig_preamble

    _NC_CACHE = nc
    return nc


def run_spmd(inputs_arr: np.ndarray, sigma_arr: np.ndarray, trace: bool = False):
    """Shard over batch, run on 8 cores, gather. Returns (out, results_obj)."""
    from concourse import bass_utils

    nc = _build_bass()

    x_full = np.ascontiguousarray(np.asarray(inputs_arr, dtype=np.float32))
    assert x_full.shape == (B, F, T), x_full.shape
    sig = np.asarray(sigma_arr, dtype=np.float32).reshape(1, 1)

    in_maps = [
        {"x": x_full[k * BPC : (k + 1) * BPC], "sig": sig} for k in range(N_CORES)
    ]
    res = bass_utils.run_bass_kernel_spmd(
        nc, in_maps, core_ids=list(range(N_CORES)), trace=trace
    )
    out = np.concatenate([r["y"] for r in res.results], axis=0)
    return out, res


def kernel(**inputs) -> np.ndarray:
    out, _ = run_spmd(inputs["inputs"], inputs["sigma"])
    return out


# revision 34
# speedup vs baseline: 1.1382x; 1.0548x over previous
"""Trainium2 Bass kernel for nn_CSAModule_47768626266174 — v9.

v7 + items 0/1 arrive through ONE casting SWDGE load (fp32->bf16,
descriptors generated on idle GPSIMD; the DMA engines move half the
bytes, shortening the stream ~720 ns), sigma through a SWDGE load
generated first.

Mathematical structure of the reference:

    S    = softmax(attn, axis=-1)                # [C, T, T]
    out  = base + sigma * einsum('bft,ct->bcf', inputs, S.mean(axis=-1))
    base = inputs.mean(-1)[:, None, :]

``S.mean(axis=-1)`` averages over the *same* axis the softmax normalizes,
so it is exactly 1/T for every (c, t) — independent of the attention
contents, the conv weights, and the labels.  Hence

    out[b, c, f] = (1 + sigma) * mean_t inputs[b, f, t]

for every class c, so the kernel only needs to read ``inputs`` once,
reduce over T, scale by (1 + sigma)/T, and broadcast over the class dim.

Sharding: data-parallel over batch B — each of the 8 cores reduces its
8-item chunk; no collectives.  Output chunks are concatenated on host.

Per-core dataflow.  The critical path is the serialized DMA data stream
(~2.1 MB at 360 GB/s), then the tail chain of the LAST-arriving chunk:
dma-sem (900) -> reduce -> matmul -> PSUM->SBUF copy -> store launch
(HWDGE 625 + DGE 650) -> transfer -> dma-sem (900).  Design notes:

  * SP's 5-instruction register preamble is skipped (-250 ns stream
    start; SP only issues DMAs / waits, which never read those regs).
  * 9 load DMAs (HWDGE is 625 ns per DMA; a 10th would stall the
    stream): items 0-6 whole, item 7 in halves so the last chunk's
    reduce is half-length.  Each load has its own semaphore (dynamic
    queues complete out of order).
  * DVE reduces items 0-5 and 7a/7b; item 6 reduces on ACT via
    activation+accumulate, so DVE is FREE exactly when 7a/7b's
    semaphores fire and the last reduces run semaphore-paced.
  * Item 7's sums are written in bf16 and its two matmuls use a bf16
    scaled-identity: 1 PE cycle/row instead of fp32's 4 (53 vs 213 ns).
    ~0.4% rounding on 1/8 of the output, vs a 2e-2 rel-err budget.
  * PE matmul order: items 0-5, then 7a(start), 6, 7b(stop).  mm7a's
    input is ready before mm6's (ACT's accum-read chain is slow) and
    mm7b's reduce lands last, so this keeps PE busy without delaying
    mm7b.  pts[6]/pts[7] are separate PSUM banks so the interleaved
    accumulation groups don't interact.  PE uses STANDALONE wait_ge
    (not waits attached to the matmul): the busy sequencer pins
    pe_busy_start so the PE p-state ramps to full speed (213 ns/mm)
    instead of restarting cold (607 ns/mm) at every matmul.
  * Copies: items 0-5 and 6 on ACT into yt; item 7 on DVE (GPSIMD may
    not touch PSUM; DVE is free after its last reduce).
  * Three HWDGE stores from SP: items 0-3 (after c3), 4-5 (after c5),
    6-7 (after c6+c7, the only store on the critical tail).  SWDGE
    prepare/trigger would launch ~1.2 us faster after the data is
    ready, but this walrus build cannot encode InstTriggerDma
    ("ISA wrong length"), so the HWDGE path it is.
  * The Bass start barrier, its dead const-tile memsets, and the
    Block-exit all-engine barrier are skipped; SP's final waits on the
    three store semaphores keep the program alive until y is in HBM.
"""

from contextlib import ExitStack

import numpy as np

B, F, T, C = 64, 128, 512, 10
N_CORES = 8
BPC = B // N_CORES  # batch items per core
H = T // 2

_NC_CACHE = None


def _build_bass():
    """Build the per-core Bass module (SPMD: same program on all cores)."""
    global _NC_CACHE
    if _NC_CACHE is not None:
        return _NC_CACHE

    import concourse.bass as bass
    import concourse.mybir as mybir

    fp32 = mybir.dt.float32
    bf16 = mybir.dt.bfloat16

    _orig_memset = bass.BassEitherVectorEngine.memset

    def _memset_skip_dead_consts(self, ap, constant):
        tensor = getattr(ap, "tensor", None)
        if tensor is not None and getattr(tensor, "name", "").startswith(
            "const-"
        ):
            return None
        return _orig_memset(self, ap, constant)

    _orig_barrier = bass.Bass.all_engine_barrier

    def _skip_barrier(self, *, sem_only: bool = False):
        return None

    _orig_preamble = bass.BassEngine.preamble

    def _preamble_skip_sp(self):
        if self.engine == mybir.EngineType.SP:
            return None
        return _orig_preamble(self)

    bass.BassEitherVectorEngine.memset = _memset_skip_dead_consts
    bass.Bass.all_engine_barrier = _skip_barrier
    bass.BassEngine.preamble = _preamble_skip_sp
    try:
        nc = bass.Bass()

        x = nc.dram_tensor("x", [BPC, F, T], fp32, kind="ExternalInput")
        sig = nc.dram_tensor("sig", [1, 1], fp32, kind="ExternalInput")
        y = nc.dram_tensor("y", [BPC, C, F], fp32, kind="ExternalOutput")

        with ExitStack() as ctx:
            e = ctx.enter_context
            # xt holds items 2..7 (fp32): item b at cols (b-2)*T.
            # Items 0/1 land as bf16 via one casting SWDGE load.
            xt = e(nc.sbuf_tensor("xt", [128, 6 * T], fp32))
            xt16 = e(nc.sbuf_tensor("xt16", [128, 2 * T], bf16))
            # sums: fp32 cols for items 2-6; bf16 cols 0,1=items 0,1 and
            # 6,7=item 7's halves.
            sums = e(nc.sbuf_tensor("sums", [128, 8], fp32))
            sums16 = e(nc.sbuf_tensor("sums16", [128, 8], bf16))
            ident_s = e(nc.sbuf_tensor("ident_s", [128, 128], fp32))
            ident16 = e(nc.sbuf_tensor("ident16", [128, 128], bf16))
            sg = e(nc.sbuf_tensor("sg", [1, 1], fp32))
            s1 = e(nc.sbuf_tensor("s1", [1, 1], fp32))
            ones_row = e(nc.sbuf_tensor("ones_row", [1, 128], fp32))
            scale_col = e(nc.sbuf_tensor("scale_col", [128, 1], fp32))
            yt = e(nc.sbuf_tensor("yt", [C, BPC * F], fp32))
            dump = e(nc.sbuf_tensor("dump", [128, T], fp32))
            # psc is allocated and immediately freed: its bank is reused by
            # pts[0].  Safe because mm0 (the first pts[0] write) waits for
            # ident_s, which waits for the scale_col copy — psc's last read.
            psc_cm = nc.psum_tensor("psc", [128, 1], fp32)
            psc = psc_cm.__enter__()
            psc_cm.__exit__(None, None, None)
            pts = [
                e(nc.psum_tensor(f"pt{b}", [C, 128], fp32))
                for b in range(BPC)
            ]

            c01_sem = e(nc.semaphore("xc01"))
            # HWDGE loads (SP): (item, xt-t0, t1, sem, engine); item b at
            # xt cols (b-2)*T.
            loads = []
            for b in range(2, 6):
                loads.append(
                    (b, (b - 2) * T, (b - 1) * T, e(nc.semaphore(f"x{b}")), "dve")
                )
            loads.append((6, 4 * T, 5 * T, e(nc.semaphore("x6")), "act"))
            x7a_sem = e(nc.semaphore("x7a"))
            x7b_sem = e(nc.semaphore("x7b"))
            loads.append((7, 5 * T, 5 * T + H, x7a_sem, "dve16"))
            loads.append((7, 5 * T + H, 6 * T, x7b_sem, "dve16"))

            sig_sem = e(nc.semaphore("sig_sem"))
            s1_sem = e(nc.semaphore("s1_sem"))
            dve_sem = e(nc.semaphore("dve_sem"))
            pool_sem = e(nc.semaphore("pool_sem"))
            pe_sem = e(nc.semaphore("pe_sem"))
            act_sem = e(nc.semaphore("act_sem"))
            act_red_sem = e(nc.semaphore("act_red_sem"))
            c67_sem = e(nc.semaphore("c67_sem"))
            ca_sem = e(nc.semaphore("ca_sem"))
            sa_sem = e(nc.semaphore("sa_sem"))
            sb_sem = e(nc.semaphore("sb_sem"))
            sc_sem = e(nc.semaphore("sc_sem"))

            # DVE reduce order (by semaphore time): r2, r3, r4, then the
            # cast pair r0/r1, r5, then item 7's halves.  Reduce spec:
            # (key, src, s0, s1col, out16?, waitsem).  sums16 cols: 0,1 =
            # items 0,1; 2,3 = item 7's halves.  sums (fp32) cols 2..5 =
            # items 2..5 (item 6 accumulates into col 6 on ACT).
            dve_reduces = [
                (2, xt, 0 * T, 1 * T, False, None),
                (3, xt, 1 * T, 2 * T, False, None),
                (4, xt, 2 * T, 3 * T, False, None),
                ("01a", xt16, 0 * T, 1 * T, True, c01_sem),
                ("01b", xt16, 1 * T, 2 * T, True, c01_sem),
                (5, xt, 3 * T, 4 * T, False, None),
                ("7a", xt, 5 * T, 5 * T + H, True, x7a_sem),
                ("7b", xt, 5 * T + H, 6 * T, True, x7b_sem),
            ]
            load_sems = {b: sem for b, t0, t1, sem, eng in loads}
            red_out16 = {"01a": 0, "01b": 1, "7a": 2, "7b": 3}
            red_out32 = {2: 2, 3: 3, 4: 4, 5: 5}
            dve_ms = {}
            n = 1
            for entry in dve_reduces:
                n += 1
                dve_ms[entry[0]] = n
            # pe_sem: 1 = psc, then the matmuls in PE program order.
            # ("f32", item, sums-col) / ("b16", item, sums16-col, start,
            # stop).  Item 7 accumulates in pts[7]; mm6 interleaves
            # between mm7a and mm7b (different PSUM banks).
            mm_order = [
                ("f32", 2, 2, True, True),
                ("f32", 3, 3, True, True),
                ("f32", 4, 4, True, True),
                ("b16", 0, 0, True, True),
                ("b16", 1, 1, True, True),
                ("f32", 5, 5, True, True),
                ("b16", 7, 2, True, False),
                ("mm6", 6, 6, True, True),
                ("b16", 7, 3, False, True),
            ]
            mm_ms = {}
            for i, ent in enumerate(mm_order):
                mm_ms[(ent[0], ent[2])] = i + 2
            MM_BY_ITEM = {
                0: mm_ms[("b16", 0)],
                1: mm_ms[("b16", 1)],
                2: mm_ms[("f32", 2)],
                3: mm_ms[("f32", 3)],
                4: mm_ms[("f32", 4)],
                5: mm_ms[("f32", 5)],
                6: mm_ms[("mm6", 6)],
                7: mm_ms[("b16", 3)],
            }
            MM7_DONE = MM_BY_ITEM[7]

            # Loads issue in the main body, before the Block's entry
            # branch: SP's first DMA starts at t=0 instead of t=50.
            for b, t0, t1, sem, eng in loads:
                nc.sync.dma_start(
                    xt[:, t0:t1], x[b, :, t0 - (b - 2) * T : t1 - (b - 2) * T]
                ).then_inc(sem, 16)

            block = e(nc.Block())

            @block.sync
            def _(sync):
                # Store A: items 0-4 (c0..c4 bump ca_sem).
                sync.dma_start(
                    y[0:5, :, :].rearrange("b c f -> c b f"),
                    yt[:, 0 : 5 * F].rearrange("c (b f) -> c b f", f=F),
                )._wait_ge(ca_sem, 5).then_inc(sa_sem, 16)
                # Store B: items 5-7 (c5/c6/c7 bump c67_sem).
                sync.dma_start(
                    y[5:8, :, :].rearrange("b c f -> c b f"),
                    yt[:, 5 * F : 8 * F].rearrange("c (b f) -> c b f", f=F),
                )._wait_ge(c67_sem, 3).then_inc(sb_sem, 16)
                sync.wait_ge(sa_sem, 16)
                sync.wait_ge(sb_sem, 16)

            @block.vector
            def _(vector):
                vector.memset(ones_row[:, :], 1.0).then_inc(dve_sem, 1)
                first = True
                for key, src_t, a0, a1, out16, wsem in dve_reduces:
                    if wsem is None:
                        wsem = load_sems[key]
                    if out16:
                        with nc.allow_low_precision(
                            "bf16 sums feed bf16 matmuls; ~0.4% rounding"
                        ):
                            col = red_out16[key]
                            red = vector.reduce_sum(
                                out=sums16[:, col : col + 1],
                                in_=src_t[:, a0:a1],
                                axis=mybir.AxisListType.X,
                            )
                    else:
                        col = red_out32[key]
                        red = vector.reduce_sum(
                            out=sums[:, col : col + 1],
                            in_=src_t[:, a0:a1],
                            axis=mybir.AxisListType.X,
                        )
                    red._wait_ge(wsem, 16).then_inc(dve_sem, 1)
                    if first:
                        first = False
                        # s1 = (1+sigma)/T right after the first reduce.
                        vector.tensor_scalar(
                            out=s1[:, :],
                            in0=sg[:, :],
                            scalar1=1.0 / T,
                            scalar2=1.0 / T,
                            op0=mybir.AluOpType.mult,
                            op1=mybir.AluOpType.add,
                        )._wait_ge(sig_sem, 16).then_inc(s1_sem, 1)
                # Items 6 and 7's PSUM -> SBUF copies (GPSIMD may not
                # touch PSUM; DVE is free after its last reduce; ACT is
                # backlogged with item 6's accum-reduce and c1/c5).
                vector.tensor_copy(
                    yt[:, 6 * F : 7 * F], pts[6][:, :]
                )._wait_ge(pe_sem, MM_BY_ITEM[6]).then_inc(c67_sem, 1)
                vector.tensor_copy(
                    yt[:, 7 * F : 8 * F], pts[7][:, :]
                )._wait_ge(pe_sem, MM7_DONE).then_inc(c67_sem, 1)

            @block.gpsimd
            def _(gpsimd):
                # SWDGE loads: sigma first (tiny; its early semaphore
                # unblocks the s1 -> ident chain), then the casting pair
                # load of items 0/1 (fp32 -> bf16: half the DMA bytes).
                gpsimd.dma_start(sg[:, :], sig[:, :]).then_inc(sig_sem, 16)
                gpsimd.dma_start(
                    xt16[:, :].rearrange("p (b t) -> p b t", b=2),
                    x[0:2, :, :].rearrange("b p t -> p b t"),
                ).then_inc(c01_sem, 16)
                # ident_s = diag((1+sigma)/T): select between a broadcast
                # of scale_col and 0.0.
                gpsimd.affine_select(
                    out=ident_s[:, :],
                    in_=scale_col[:, :].broadcast_to((128, 128)),
                    compare_op=mybir.AluOpType.is_equal,
                    fill=0.0,
                    base=0,
                    pattern=[[-1, 128]],
                    channel_multiplier=1,
                )._wait_ge(act_sem, 1).then_inc(pool_sem, 1)
                gpsimd.tensor_copy(ident16[:, :], ident_s[:, :])

            @block.scalar
            def _(scalar):
                # scale_col = (1+sigma)/T on all partitions, via psc.
                scalar.activation(
                    out=scale_col[:, :],
                    in_=psc[:, :],
                    func=mybir.ActivationFunctionType.Copy,
                )._wait_ge(pe_sem, 1).then_inc(act_sem, 1)
                # Copies c2, c3, c4 (early, reduce-paced), then c0, then
                # item 6's accum-reduce, then c1 and c5 as their matmuls
                # land.  c6/c7 run on DVE.
                for bidx in (2, 3, 4):
                    scalar.activation(
                        out=yt[:, bidx * F : (bidx + 1) * F],
                        in_=pts[bidx][:, :],
                        func=mybir.ActivationFunctionType.Copy,
                    )._wait_ge(pe_sem, MM_BY_ITEM[bidx]).then_inc(ca_sem, 1)
                scalar.activation(
                    out=yt[:, 0 * F : 1 * F],
                    in_=pts[0][:, :],
                    func=mybir.ActivationFunctionType.Copy,
                )._wait_ge(pe_sem, MM_BY_ITEM[0]).then_inc(ca_sem, 1)
                # Item 6's reduce: activation+accumulate into sums col 6.
                scalar.activation(
                    out=dump[:, 0:T],
                    in_=xt[:, 4 * T : 5 * T],
                    func=mybir.ActivationFunctionType.Copy,
                    accum_out=sums[:, 6:7],
                )._wait_ge(load_sems[6], 16).then_inc(act_red_sem, 1)
                scalar.activation(
                    out=yt[:, 1 * F : 2 * F],
                    in_=pts[1][:, :],
                    func=mybir.ActivationFunctionType.Copy,
                )._wait_ge(pe_sem, MM_BY_ITEM[1]).then_inc(ca_sem, 1)
                scalar.activation(
                    out=yt[:, 5 * F : 6 * F],
                    in_=pts[5][:, :],
                    func=mybir.ActivationFunctionType.Copy,
                )._wait_ge(pe_sem, MM_BY_ITEM[5]).then_inc(c67_sem, 1)

            @block.tensor
            def _(tensor):
                # Standalone waits keep PE.SEQ occupied between matmuls,
                # pinning pe_busy_start so the p-state ramps to full speed.
                # psc[p, 0] = (1+sigma)/T on every partition (K=1 matmul).
                tensor.wait_ge(dve_sem, 1)  # ones_row
                tensor.wait_ge(s1_sem, 1)
                tensor.matmul(
                    psc[:, :], ones_row[:, :], s1[:, :], start=True, stop=True
                ).then_inc(pe_sem, 1)
                tensor.wait_ge(pool_sem, 1)  # ident_s ready
                red_key = {
                    ("b16", 0): "01a",
                    ("b16", 1): "01b",
                    ("b16", 2): "7a",
                    ("b16", 3): "7b",
                    ("f32", 2): 2,
                    ("f32", 3): 3,
                    ("f32", 4): 4,
                    ("f32", 5): 5,
                }
                for kind, b, col, is_start, is_stop in mm_order:
                    if kind == "mm6":
                        tensor.wait_ge(act_red_sem, 1)
                        lhsT = sums[:, 6:7].broadcast_to((128, C))
                        rhs = ident_s[:, :]
                    else:
                        tensor.wait_ge(dve_sem, dve_ms[red_key[(kind, col)]])
                        if kind == "b16":
                            lhsT = sums16[:, col : col + 1].broadcast_to(
                                (128, C)
                            )
                            rhs = ident16[:, :]
                        else:
                            lhsT = sums[:, col : col + 1].broadcast_to(
                                (128, C)
                            )
                            rhs = ident_s[:, :]
                    tensor.matmul(
                        pts[b][:, :],
                        lhsT,
                        rhs,
                        start=is_start,
                        stop=is_stop,
                    ).then_inc(pe_sem, 1)

    finally:
        bass.BassEitherVectorEngine.memset = _orig_memset
        bass.Bass.all_engine_barrier = _orig_barrier
        bass.BassEngine.preamble = _orig_preamble

    _NC_CACHE = nc
    return nc


def run_spmd(inputs_arr: np.ndarray, sigma_arr: np.ndarray, trace: bool = False):
    """Shard over batch, run on 8 cores, gather. Returns (out, results_obj)."""
    from concourse import bass_utils

    nc = _build_bass()

    x_full = np.ascontiguousarray(np.asarray(inputs_arr, dtype=np.float32))
    assert x_full.shape == (B, F, T), x_full.shape
    sig = np.asarray(sigma_arr, dtype=np.float32).reshape(1, 1)

    in_maps = [
        {"x": x_full[k * BPC : (k + 1) * BPC], "sig": sig} for k in range(N_CORES)
    ]
    res = bass_utils.run_bass_kernel_spmd(
        nc, in_maps, core_ids=list(range(N_CORES)), trace=trace
    )
    out = np.concatenate([r["y"] for r in res.results], axis=0)
    return out, res


def kernel(**inputs) -> np.ndarray:
    out, _ = run_spmd(inputs["inputs"], inputs["sigma"])
    return out
